# revision 6
# baseline (speedup 1.0000x reference)
"""Trainium2 Bass kernel for nn_Decoder_Flow (CNF decoder with Hutchinson
divergence, RK4 integration).

Strategy (data-parallel over the B*T=32 context axis, 4 contexts/core on 8
cores):
  - "strip" layout: per-core point clouds live at SBUF partitions 32n+d
    (n = local context 0..3, d = xyz), K=1024 points on the free axis.
  - primal MLP (3->256->256->256) in float32r (full-rate fp32 matmul mode);
    final 256->3 layer + entire JVP/tangent path in bf16.
  - hypernet gates/biases for all 40 RK4 stage evals precomputed on host and
    folded into per-partition activation scale/bias vectors + matmul masks.
  - tangent update fused into one custom DVE op:
        tz = (p^2 * (-g) + g) * z_t  ==  g*(1-p^2)*z_t
  - divergence e^T J e reduced over d by a small mask matmul whose weights
    carry -dt/6 * rk_weight * gate4, accumulated into lp via vector adds.
"""

import sys

import numpy as np
import ml_dtypes

try:
    import concourse  # noqa: F401
except ImportError:  # pragma: no cover
    sys.path.insert(0, "/opt/trn_rl_repo")

import concourse.bacc as bacc
import concourse.mybir as mybir
from concourse import tile
from concourse.bass_utils import run_bass_kernel_spmd

F32R = mybir.dt.float32r
F32 = mybir.dt.float32
BF16 = mybir.dt.bfloat16
BF = ml_dtypes.bfloat16

D = 3
ZDIM = 128
HID = 256
B, T, K = 4, 8, 1024
NCTX = B * T  # 32
NCORES = 8
NPC = NCTX // NCORES  # 4 contexts per core
NUM_STEPS = 10
T_END = 1.0
DT = T_END / NUM_STEPS
CHUNK = 512
NCH = K // CHUNK  # 2

# gate-blob columns per dyn eval:
#   layers 0..2: (l, n, dtile) -> 3 cols [g, -g, g*b+bias]
#   layer 3 (strip layout): 4 cols [ys_scale, ys_bias, ya_scale, ya_bias]
GC_L = 3  # cols per (l, n, dtile)
GC_DYN = 3 * NPC * 2 * GC_L + 4  # 76

_RK_C = [0.5 * DT, 0.5 * DT, DT]  # stage-input scales (stages 0..2)
_RK_W = [DT / 6.0, DT / 3.0, DT / 3.0, DT / 6.0]  # accumulation weights


# --------------------------------------------------------------------------
# custom DVE op: out = (sq(in0) * s0 + s1) * in1
# --------------------------------------------------------------------------
_JVP_OP = None


def _get_jvp_op():
    global _JVP_OP
    if _JVP_OP is not None:
        return _JVP_OP
    import concourse.dve_ops as dv
    from concourse.dve_spec import Spec, Src0, Src1, C0, C1, sq, lower, _has_src1
    from concourse.dve_uop import DveOpSpec

    name = "TANH_JVP_ANT"
    for op in dv.OPS:
        if op.name == name:
            _JVP_OP = op
            return op
    spec = Spec(
        body=(sq(Src0) * C0 + C1) * Src1,
        reference=lambda in0, in1, c0, c1, c2: (
            np.square(in0.astype(np.float32)) * c0 + c1
        )
        * in1,
    )
    row = dv._CUSTOM_DVE_ROW_BASE + len(dv.OPS)
    dv._SUB_OPCODE_FOR_NAME[name] = row
    shas = {}
    for ver in ("v3", "v4"):
        tmp = DveOpSpec(
            name=name, opcode=row, uops=lower(spec, ver=ver), rd1_en=_has_src1(spec)
        )
        shas[ver] = tmp.sha(ver)
    op = dv.DveOp(name, spec, subdim=False, uops_sha=shas)
    dv.OPS.append(op)
    dv.CUSTOM_DVE_SPECS[name] = spec
    _JVP_OP = op
    return op


# --------------------------------------------------------------------------
# host-side preparation
# --------------------------------------------------------------------------
def _host_prep(x, h, e, params, num_steps=NUM_STEPS):
    """Build per-core input maps."""
    dt = T_END / num_steps
    rk_c = [0.5 * dt, 0.5 * dt, dt]
    rk_w = [dt / 6.0, dt / 3.0, dt / 3.0, dt / 6.0]
    x = np.asarray(x, np.float32).reshape(NCTX, K, D)
    h2 = np.asarray(h, np.float32).reshape(NCTX, ZDIM)
    e = np.asarray(e, np.float32).reshape(NCTX, K, D)
    W = [np.asarray(p["W"], np.float32) for p in params]
    bvec = [np.asarray(p["b"], np.float32) for p in params]
    Wg = [np.asarray(p["Wg"], np.float32) for p in params]
    bg = [np.asarray(p["bg"], np.float32) for p in params]
    Wb = [np.asarray(p["Wb"], np.float32) for p in params]

    n_dyn = num_steps * 4
    times = []
    for i in range(num_steps):
        t0 = i * dt
        times.append((t0, t0 + 0.5 * dt, t0 + 0.5 * dt, t0 + dt))

    # weight blobs (shared across cores)
    # wf (f32r): W1 strips [128,256] | W2 [128,512] | W3 [128,512]
    wf = np.zeros((128, 256 + 512 + 512), np.float32)
    for n in range(NPC):
        for dt_ in range(2):
            wf[32 * n : 32 * n + 3, 128 * dt_ : 128 * dt_ + 128] = W[0][
                :, 128 * dt_ : 128 * dt_ + 128
            ]
    for li, base in ((1, 256), (2, 768)):
        for kt in range(2):
            for dt_ in range(2):
                c = base + 128 * (2 * kt + dt_)
                wf[:, c : c + 128] = W[li][
                    128 * kt : 128 * kt + 128, 128 * dt_ : 128 * dt_ + 128
                ]
    # wb (bf16): W1strips [*,256] | W2 [*,512] | W3 [*,512] | W4pad [*,64]
    wbf = np.zeros((128, 256 + 512 + 512 + 64), np.float32)
    wbf[:, 0:1280] = wf
    for kt in range(2):
        c = 1280 + 32 * kt
        wbf[:, c : c + 3] = W[3][128 * kt : 128 * kt + 128, :]
    wb = wbf.astype(BF)

    in_maps = []
    for core in range(NCORES):
        ns = [core * NPC + n for n in range(NPC)]
        ys0 = np.zeros((128, K), np.float32)
        ef32 = np.zeros((128, K), np.float32)
        for n in range(NPC):
            for d in range(D):
                ys0[32 * n + d] = x[ns[n], :, d]
                ef32[32 * n + d] = e[ns[n], :, d]
        ebf = ef32.astype(BF)

        gates = np.zeros((128, n_dyn * GC_DYN), np.float32)
        maskf = np.zeros((128, n_dyn * 4), np.float32)
        for dyn in range(n_dyn):
            i, s = divmod(dyn, 4)
            t = times[i][s]
            base = dyn * GC_DYN
            for n in range(NPC):
                tc = np.concatenate([[t], h2[ns[n]]]).astype(np.float32)
                for li in range(3):
                    gate = 1.0 / (1.0 + np.exp(-(tc @ Wg[li] + bg[li])))
                    bias = tc @ Wb[li]
                    bc = gate * bvec[li] + bias
                    for dt_ in range(2):
                        c = base + ((li * NPC + n) * 2 + dt_) * GC_L
                        sl = slice(128 * dt_, 128 * dt_ + 128)
                        gates[:, c + 0] = gate[sl]
                        gates[:, c + 1] = -gate[sl]
                        gates[:, c + 2] = bc[sl]
                # layer 3
                gate4 = 1.0 / (1.0 + np.exp(-(tc @ Wg[3] + bg[3])))
                bias4 = tc @ Wb[3]
                c4 = gate4 * bvec[3] + bias4
                c = base + 3 * NPC * 2 * GC_L
                st = slice(32 * n, 32 * n + 3)
                if s < 3:
                    gates[st, c + 0] = rk_c[s] * gate4
                    gates[st, c + 1] = rk_c[s] * c4
                gates[st, c + 2] = rk_w[s] * gate4
                gates[st, c + 3] = rk_w[s] * c4
                maskf[st, dyn * 4 + n] = -rk_w[s] * gate4
        in_maps.append(
            {
                "ys0": ys0,
                "ebf": ebf,
                "ef32": ef32,
                "wf": wf,
                "wb": wb,
                "gates": gates,
                "maskf": maskf,
            }
        )
    return in_maps


# --------------------------------------------------------------------------
# device kernel builder
# --------------------------------------------------------------------------
def _build(num_steps=NUM_STEPS):
    jvp = _get_jvp_op()
    n_dyn = num_steps * 4
    nc = bacc.Bacc()
    ys0_d = nc.dram_tensor("ys0", [128, K], F32R, kind="ExternalInput")
    ebf_d = nc.dram_tensor("ebf", [128, K], BF16, kind="ExternalInput")
    ef32_d = nc.dram_tensor("ef32", [128, K], F32, kind="ExternalInput")
    wf_d = nc.dram_tensor("wf", [128, 1280], F32R, kind="ExternalInput")
    wb_d = nc.dram_tensor("wb", [128, 1344], BF16, kind="ExternalInput")
    gates_d = nc.dram_tensor(
        "gates", [128, n_dyn * GC_DYN], F32, kind="ExternalInput"
    )
    maskf_d = nc.dram_tensor("maskf", [128, n_dyn * 4], F32R, kind="ExternalInput")
    yout_d = nc.dram_tensor("yout", [128, K], F32, kind="ExternalOutput")
    lpout_d = nc.dram_tensor("lpout", [NPC, K], F32, kind="ExternalOutput")

    with tile.TileContext(nc) as tc:
        with (
            tc.tile_pool(name="const", bufs=1) as constp,
            tc.tile_pool(name="pf", bufs=18) as pfp,
            tc.tile_pool(name="pb", bufs=9) as pbp,
            tc.tile_pool(name="tz", bufs=18) as tzp,
            tc.tile_pool(name="ysp", bufs=2) as ysp,
            tc.tile_pool(name="qp", bufs=3) as qp,
            tc.tile_pool(name="zp", bufs=2, space="PSUM") as zpp,
            tc.tile_pool(name="zt", bufs=3, space="PSUM") as ztp,
            tc.tile_pool(name="ps4", bufs=2, space="PSUM") as ps4p,
            tc.tile_pool(name="dvp", bufs=1, space="PSUM") as dvp,
        ):
            y_sb = constp.tile([128, K], F32R, tag="y_sb")
            yacc = constp.tile([128, K], F32, tag="yacc")
            lp_sb = constp.tile([NPC, K], F32, tag="lp_sb")
            ebf = constp.tile([128, K], BF16, tag="ebf")
            ef32 = constp.tile([128, K], F32, tag="ef32")
            wf = constp.tile([128, 1280], F32R, tag="wf")
            wb = constp.tile([128, 1344], BF16, tag="wb")
            gates = constp.tile([128, n_dyn * GC_DYN], F32, tag="gates")
            maskf = constp.tile([128, n_dyn * 4], F32R, tag="maskf")

            nc.sync.dma_start(y_sb[:], ys0_d[:])
            nc.sync.dma_start(ebf[:], ebf_d[:])
            nc.sync.dma_start(ef32[:], ef32_d[:])
            nc.sync.dma_start(wf[:], wf_d[:])
            nc.sync.dma_start(wb[:], wb_d[:])
            nc.sync.dma_start(gates[:], gates_d[:])
            nc.sync.dma_start(maskf[:], maskf_d[:])
            nc.vector.memzero(lp_sb[:])

            def w_f(li, kt, dt_):
                if li == 0:
                    return None
                base = 256 if li == 1 else 768
                c = base + 128 * (2 * kt + dt_)
                return wf[:, c : c + 128]

            def w_b(li, kt, dt_):
                base = 256 if li == 1 else 768
                c = base + 128 * (2 * kt + dt_)
                return wb[:, c : c + 128]

            def gcol(dyn, idx):
                c = dyn * GC_DYN + idx
                return gates[:, c : c + 1]

            ys_prev = None
            for dyn in range(n_dyn):
                s = dyn % 4
                rhs_p = y_sb if s == 0 else ys_prev
                p_prev = None  # [n][kt] tiles
                t_prev = None
                for li in range(3):
                    pdt = F32R if li < 2 else BF16
                    ppool = pfp if li < 2 else pbp
                    p_new = []
                    t_new = []
                    for n in range(NPC):
                        p_nd = []
                        t_nd = []
                        for dt_ in range(2):
                            cbase = (dyn * GC_DYN) + ((li * NPC + n) * 2 + dt_) * GC_L
                            g_ap = gates[:, cbase : cbase + 1]
                            ng_ap = gates[:, cbase + 1 : cbase + 2]
                            bc_ap = gates[:, cbase + 2 : cbase + 3]
                            pt = ppool.tile([128, K], pdt, tag="pf" if li < 2 else "pb")
                            tz = tzp.tile([128, K], BF16, tag="tz")
                            for ch in range(NCH):
                                cs = slice(CHUNK * ch, CHUNK * ch + CHUNK)
                                zp = zpp.tile([128, CHUNK], F32, tag="zp")
                                zt = ztp.tile([128, CHUNK], F32, tag="zt")
                                if li == 0:
                                    st = slice(32 * n, 32 * n + 3)
                                    nc.tensor.matmul(
                                        zp[:],
                                        wf[st, 128 * dt_ : 128 * dt_ + 128],
                                        rhs_p[st, cs],
                                        start=True,
                                        stop=True,
                                        tile_position=(32 * n, 0),
                                    )
                                    nc.tensor.matmul(
                                        zt[:],
                                        wb[st, 128 * dt_ : 128 * dt_ + 128],
                                        ebf[st, cs],
                                        start=True,
                                        stop=True,
                                        tile_position=(32 * n, 0),
                                    )
                                else:
                                    for kt in range(2):
                                        nc.tensor.matmul(
                                            zp[:],
                                            w_f(li, kt, dt_),
                                            p_prev[n][kt][:, cs],
                                            start=(kt == 0),
                                            stop=(kt == 1),
                                        )
                                    for kt in range(2):
                                        nc.tensor.matmul(
                                            zt[:],
                                            w_b(li, kt, dt_),
                                            t_prev[n][kt][:, cs],
                                            start=(kt == 0),
                                            stop=(kt == 1),
                                        )
                                nc.scalar.activation(
                                    pt[:, cs],
                                    zp[:],
                                    mybir.ActivationFunctionType.Tanh,
                                    bias=bc_ap,
                                    scale=g_ap,
                                )
                                in0 = pt[:, cs]
                                if pdt == F32R:
                                    in0 = in0.bitcast(F32)
                                nc.vector._custom_dve(
                                    jvp,
                                    out=tz[:, cs],
                                    in0=in0,
                                    in1=zt[:],
                                    s0=ng_ap,
                                    s1=g_ap,
                                )
                            p_nd.append(pt)
                            t_nd.append(tz)
                        p_new.append(p_nd)
                        t_new.append(t_nd)
                    p_prev, t_prev = p_new, t_new

                # layer 4 + state updates + divergence
                c4 = dyn * GC_DYN + 3 * NPC * 2 * GC_L
                if s < 3:
                    ys_new = ysp.tile([128, K], F32R, tag="ys")
                else:
                    ys_new = None
                for ch in range(NCH):
                    cs = slice(CHUNK * ch, CHUNK * ch + CHUNK)
                    p4 = ps4p.tile([128, CHUNK], F32, tag="ps4")
                    for n in range(NPC):
                        for kt in range(2):
                            nc.tensor.matmul(
                                p4[32 * n : 32 * n + 32, :],
                                wb[:, 1280 + 32 * kt : 1280 + 32 * kt + 32],
                                p_prev[n][kt][:, cs],
                                start=(kt == 0),
                                stop=(kt == 1),
                                tile_position=(0, 32 * n),
                            )
                    if s < 3:
                        nc.vector.affine_then_add(
                            ys_new[:, cs],
                            p4[:],
                            y_sb[:, cs].bitcast(F32),
                            gcol(dyn, c4 - dyn * GC_DYN + 0),
                            gcol(dyn, c4 - dyn * GC_DYN + 1),
                        )
                    acc_in = y_sb[:, cs].bitcast(F32) if s == 0 else yacc[:, cs]
                    acc_out = yacc[:, cs] if s < 3 else y_sb[:, cs]
                    nc.vector.affine_then_add(
                        acc_out,
                        p4[:],
                        acc_in,
                        gates[:, c4 + 2 : c4 + 3],
                        gates[:, c4 + 3 : c4 + 4],
                    )
                    t4 = ps4p.tile([128, CHUNK], F32, tag="ps4")
                    for n in range(NPC):
                        for kt in range(2):
                            nc.tensor.matmul(
                                t4[32 * n : 32 * n + 32, :],
                                wb[:, 1280 + 32 * kt : 1280 + 32 * kt + 32],
                                t_prev[n][kt][:, cs],
                                start=(kt == 0),
                                stop=(kt == 1),
                                tile_position=(0, 32 * n),
                            )
                    q = qp.tile([128, CHUNK], F32R, tag="q")
                    nc.vector.tensor_mul(q[:], t4[:], ef32[:, cs])
                    dv_ps = dvp.tile([NPC, CHUNK], F32, tag="dv")
                    nc.tensor.matmul(
                        dv_ps[:],
                        maskf[:, dyn * 4 : dyn * 4 + 4],
                        q[:],
                        start=True,
                        stop=True,
                    )
                    nc.vector.tensor_add(lp_sb[:, cs], lp_sb[:, cs], dv_ps[:])
                ys_prev = ys_new

            nc.sync.dma_start(yout_d[:], y_sb[:].bitcast(F32))
            nc.sync.dma_start(lpout_d[:], lp_sb[:])
    nc.finalize()
    return nc


_NC_CACHE = {}


def _get_nc(num_steps=NUM_STEPS):
    if num_steps not in _NC_CACHE:
        _NC_CACHE[num_steps] = _build(num_steps)
    return _NC_CACHE[num_steps]


# --------------------------------------------------------------------------
# public entry point
# --------------------------------------------------------------------------
def kernel(x, h, e, params, _num_steps=NUM_STEPS, _trace=False):
    nc = _get_nc(_num_steps)
    in_maps = _host_prep(x, h, e, params, num_steps=_num_steps)
    res = run_bass_kernel_spmd(nc, in_maps, list(range(NCORES)), trace=_trace)
    x0 = np.zeros((NCTX, K, D), np.float32)
    lp = np.zeros((NCTX, K, 1), np.float32)
    for core in range(NCORES):
        r = res.results[core]
        for n in range(NPC):
            g = core * NPC + n
            for d in range(D):
                x0[g, :, d] = r["yout"][32 * n + d]
            lp[g, :, 0] = r["lpout"][n]
    kernel.last_result = res
    return x0, lp


if __name__ == "__main__":
    # quick self-check with random data against a numpy reference
    import reference

    inputs = reference.setup_inputs()
    steps = int(sys.argv[1]) if len(sys.argv) > 1 else 1
    import time

    t0 = time.time()
    out = kernel(
        np.asarray(inputs["x"]),
        np.asarray(inputs["h"]),
        np.asarray(inputs["e"]),
        inputs["params"],
        _num_steps=steps,
    )
    print(f"ran in {time.time() - t0:.1f}s (steps={steps})")
